# revision 6
# baseline (speedup 1.0000x reference)
"""Multi-head attention with relative position bias (music-transformer skew)
on 8 Trainium2 NeuronCores — v2, instruction/DMA-lean redesign.

Sharding: core c handles batch c//4 and heads 4*(c%4)..+3 (2 pairs of 2
heads). Partial y outputs (bf16) are summed on the host.

Key changes vs v1:
- Padded DRAM scratch per pair [N, 2*1152]: row = [h0 band 1152 | h1 band
  1152], cols 1024..1152 hold 0.0 so the skew-read spill region comes back
  pre-masked (no tri01 multiply, no wrap-chunk writes). One write + one
  skew-read DMA per (pair, block) covering both heads.
- gpsimd cast-DMAs load x/w/e1 as bf16 (half the DMA bytes, no cast ops).
- 1/sqrt(dh) folded into the exp activations via scale=.
- 2-bank [128,1024] PSUM tiles halve drain instruction counts.
- Pairs advance block-by-block in one merged software pipeline so PE never
  drains; p-state stays at 2.4GHz.
- y written once (pairs combined in PSUM) as bf16: 2MB instead of 8MB f32.
"""

import numpy as np

import concourse.bass as bass
import concourse.tile as tile
from concourse import bacc, mybir
from concourse.bass import ds, ts
from concourse.bass_utils import run_bass_kernel_spmd
from concourse.masks import make_identity

B, N, D, H, DH, DO = 2, 1024, 1024, 16, 64, 1024
HPC = 4              # heads per core
COLS = HPC * DH      # 256 projection columns per core
NB = N // 128        # 8 row blocks
KT = D // 128        # 8 contraction tiles
BAND = 1152          # per-head scratch row: 1024 real + 128 pad
RS = 2 * BAND        # scratch row stride (2 heads)
F32 = mybir.dt.float32
BF16 = mybir.dt.bfloat16
SCALE = 1.0 / np.sqrt(DH)
EXP = mybir.ActivationFunctionType.Exp
MULT = mybir.AluOpType.mult


def _body(tc):
    nc = tc.nc
    xb = nc.dram_tensor("xb", [N, D], BF16, kind="ExternalInput")
    wq = nc.dram_tensor("wq", [D, COLS], BF16, kind="ExternalInput")
    wk = nc.dram_tensor("wk", [D, COLS], BF16, kind="ExternalInput")
    wv = nc.dram_tensor("wv", [D, COLS], BF16, kind="ExternalInput")
    wo = nc.dram_tensor("wo", [COLS, DO], BF16, kind="ExternalInput")
    e1 = nc.dram_tensor("e1", [N, DH], BF16, kind="ExternalInput")
    qs = [nc.dram_tensor(f"qs{pr}", [N, RS], BF16) for pr in range(2)]
    ys = nc.dram_tensor("ys", [N, DO], BF16, kind="ExternalOutput")

    from contextlib import ExitStack
    ctx = ExitStack()
    singles = ctx.enter_context(tc.tile_pool(name="singles", bufs=1))
    persist = ctx.enter_context(tc.tile_pool(name="persist", bufs=1))
    work = ctx.enter_context(tc.tile_pool(name="work", bufs=2))
    psA = ctx.enter_context(tc.tile_pool(name="psA", bufs=6, space="PSUM"))
    psT = ctx.enter_context(tc.tile_pool(name="psT", bufs=2, space="PSUM"))

    ident = singles.tile([128, 128], BF16, tag="ident", name="ident")
    make_identity(nc, ident)

    # act-table warmup: load the Exp table while everything else is idle
    warm = singles.tile([128, 1], F32, tag="warm", name="warm")
    nc.scalar.activation(warm, warm, EXP)

    # ---- input loads: gpsimd SWDGE cast-DMAs (f32 DRAM -> bf16 SBUF) ----
    xl = [persist.tile([128, 1, D], BF16, tag=f"xl{q}", name=f"xl{q}")
          for q in range(8)]
    wql = persist.tile([128, KT, COLS], BF16, tag="wql", name="wql")
    e1l = singles.tile([128, 8, DH], BF16, tag="e1l", name="e1l")
    wkl = persist.tile([128, KT, COLS], BF16, tag="wkl", name="wkl")
    wvl = persist.tile([128, KT, COLS], BF16, tag="wvl", name="wvl")
    wol = persist.tile([128, 2, DO], BF16, tag="wol", name="wol")

    for q in range(4):
        nc.sync.dma_start(
            out=xl[q],
            in_=xb.rearrange("(nt p) d -> p nt d", p=128)[:, ds(q, 1), :])
    nc.sync.dma_start(
        out=wql, in_=wq.rearrange("(kt p) c -> p kt c", p=128))
    nc.sync.dma_start(
        out=e1l, in_=e1.rearrange("(nt p) d -> p nt d", p=128))
    for q in range(4, 8):
        nc.sync.dma_start(
            out=xl[q],
            in_=xb.rearrange("(nt p) d -> p nt d", p=128)[:, ds(q, 1), :])
    nc.sync.dma_start(
        out=wkl, in_=wk.rearrange("(kt p) c -> p kt c", p=128))
    nc.sync.dma_start(
        out=wvl, in_=wv.rearrange("(kt p) c -> p kt c", p=128))
    nc.sync.dma_start(
        out=wol, in_=wo.rearrange("(ct p) c -> p ct c", p=128))

    # ---- persistent SBUF ----
    xTt = persist.tile([128, KT, N], BF16, tag="xT", name="xT")
    xT = [xTt[:, k, :] for k in range(KT)]
    qt = [persist.tile([128, N], BF16, tag=f"qt{p}", name=f"qt{p}")
          for p in range(2)]
    kt_sb = [persist.tile([128, N], BF16, tag=f"kt{p}", name=f"kt{p}")
             for p in range(2)]
    v_sb = persist.tile([128, NB, COLS], BF16, tag="vsb", name="vsb")
    e1t = singles.tile([128, N], BF16, tag="e1t", name="e1t")
    ot = [persist.tile([128, N], BF16, tag=f"ot{p}", name=f"ot{p}")
          for p in range(2)]
    pts = [[persist.tile([128, NB, N], BF16, tag=f"pts{p}{h}",
                         name=f"pts{p}{h}") for h in range(2)]
           for p in range(2)]
    # qec staging for the scratch write: manual 3-buffer rotation so the pad
    # columns [1024,1152) can be zeroed exactly once per physical buffer.
    QECB = 4
    qec_l = [persist.tile([128, 2, BAND], BF16, tag=f"qec{b}", name=f"qec{b}")
             for b in range(QECB)]
    for b in range(QECB):
        for hs in range(2):
            nc.vector.memset(qec_l[b][:, hs, ds(N, 128)], -1e9)

    # ---- x transposes (per half: 8 k-chunks packed 2-per-psT-tile) ----
    def xpose_block(nb):
        # 8 transposes (one per k-chunk) for row block nb
        pt_t = psT.tile([128, 1024], BF16, tag="pt", name="pt")
        for k in range(KT):
            nc.tensor.transpose(
                pt_t[:, ts(k, 128)], xl[nb][:, 0, ts(k, 128)], ident)
        nc.vector.tensor_copy(
            xTt[:, :, ts(nb, 128)],
            pt_t.rearrange("p (k c) -> p k c", k=KT))

    # p-state prewarm: dummy transposes so the PE is at full clock when x
    # lands (ramp needs ~3us of continuous busy)
    warmps = psT.tile([128, 1024], BF16, tag="pt", name="warmps")
    for _ in range(12):
        nc.tensor.transpose(warmps[:, ds(0, 128)], ident, ident)

    def emit_proj(w_all, dest, ct, nh, drain):
        ps = psA.tile([128, 512], F32, tag="a", name="psqk")
        for k in range(KT):
            nc.tensor.matmul(
                ps[:, ds(0, 512)], w_all[:, k, ts(ct, 128)],
                xT[k][:, ds(512 * nh, 512)],
                start=(k == 0), stop=(k == KT - 1))
        if drain == "act":
            nc.scalar.copy(dest[ct][:, ds(512 * nh, 512)], ps[:, ds(0, 512)])
        else:
            nc.vector.tensor_copy(
                dest[ct][:, ds(512 * nh, 512)], ps[:, ds(0, 512)])

    def emit_v(mp):
        ps = psA.tile([128, 512], F32, tag="a", name="psv")
        for sub in range(2):
            nb = 2 * mp + sub
            for k in range(KT):
                nc.tensor.matmul(
                    ps[:, ds(256 * sub, 256)],
                    xT[k][:, ts(nb, 128)], wvl[:, k, :],
                    start=(k == 0), stop=(k == KT - 1))
        nc.vector.tensor_copy(
            v_sb[:, ds(2 * mp, 2), :].rearrange("p a b -> p (a b)"),
            ps[:, ds(0, 512)])


    # ---- skew scratch plumbing ----
    def emit_qe(pr, b):
        """expR band for block b, both heads -> qec -> DRAM scratch."""
        lo = 896 - 128 * b
        width = N - lo
        qec = qec_l[qe_ctr[0] % QECB]
        qe_ctr[0] += 1
        nch = (width + 511) // 512
        for hs in range(2):
            base = 64 * hs
            for c in range(nch):
                cw = min(512, width - 512 * c)
                ps = psA.tile([128, 512], F32, tag="a", name="psq")
                nc.tensor.matmul(
                    ps[:, ds(0, cw)],
                    qt[pr][base:base + 64, ts(b, 128)],
                    e1t[base:base + 64, ds(lo + 512 * c, cw)],
                    start=True, stop=True)
                # raw (un-exp'd) drains: hs0 on DVE, hs1 on ACT
                if hs == 0:
                    nc.vector.tensor_copy(
                        qec[:, 0, ds(lo + 512 * c, cw)], ps[:, ds(0, cw)])
                else:
                    nc.scalar.copy(
                        qec[:, 1, ds(lo + 512 * c, cw)], ps[:, ds(0, cw)])
        nc.scalar.dma_start(
            out=bass.AP(
                tensor=qs[pr][:, :].tensor,
                offset=128 * b * RS + lo,
                ap=[[RS, 128], [BAND, 2], [1, BAND - lo]],
            ),
            in_=qec[:, :, ds(lo, BAND - lo)])

    def emit_read(pr, b, rel):
        width = 128 * (b + 1)
        nc.gpsimd.dma_start(
            out=rel[:, :, ds(0, width)],
            in_=bass.AP(
                tensor=qs[pr][:, :].tensor,
                offset=128 * b * (RS - 1) + (N - 1),
                ap=[[RS - 1, 128], [BAND, 2], [1, width]],
            ))

    qe_ctr = [0]
    state = {}
    deferred = []

    def emit_s(pr, b, rel):
        """S matmuls + exp + P=es*rel (+Z) + 1/Z diag for block b."""
        width = 128 * (b + 1)
        nch = (width + 511) // 512
        for hs in range(2):
            base = 64 * hs
            p_sb = work.tile([128, N], BF16, tag=f"p{hs}", name="p", bufs=2)
            zs = []
            for c in range(nch):
                cw = min(512, width - 512 * c)
                ps = psA.tile([128, 512], F32, tag="a", name="pss")
                nc.tensor.matmul(
                    ps[:, ds(0, cw)],
                    qt[pr][base:base + 64, ts(b, 128)],
                    kt_sb[pr][base:base + 64, ds(512 * c, cw)],
                    start=True, stop=False)
                # accumulate the skew-read rel bias into the S PSUM on the
                # PE (identity-matmul add): one fused exp(scale*(S+R)) later
                nc.tensor.matmul(
                    ps[:, ds(0, cw)], ident,
                    rel[:, hs, ds(512 * c, cw)],
                    start=False, stop=True)
                z = work.tile([128, 1], F32, tag=f"z{hs}{c}", name="z",
                              bufs=2)
                nc.scalar.activation(
                    p_sb[:, ds(512 * c, cw)], ps[:, ds(0, cw)],
                    EXP, scale=SCALE, accum_out=z)
                zs.append(z)
            if nch == 2:
                zt = work.tile([128, 1], F32, tag=f"zt{hs}", name="zt",
                               bufs=2)
                nc.vector.tensor_tensor(
                    out=zt, in0=zs[0], in1=zs[1], op=mybir.AluOpType.add)
            else:
                zt = zs[0]
            r = work.tile([128, 1], F32, tag=f"r{hs}", name="r", bufs=2)
            nc.vector.reciprocal(r, zt)
            # normalize rows on DVE (4x perf mode: ~0.26 cyc/elem)
            pn = work.tile([128, N], BF16, tag=f"pn{hs}", name="pn", bufs=3)
            nc.vector.tensor_scalar_mul(
                pn[:, ds(0, width)], p_sb[:, ds(0, width)], r)
            state[(pr, b, hs)] = pn

    def flush_deferred():
        pass

    def emit_pt(pr, b):
        """P^T = P.T @ diag(1/Z) for block b (both heads)."""
        for hs in range(2):
            pn = state.pop((pr, b, hs))
            jn = b + 1
            psp = psT.tile([128, 1024], BF16, tag="pt", name="psp")
            for jj in range(jn):
                nc.tensor.transpose(
                    psp[:, ts(jj, 128)], pn[:, ts(jj, 128)], ident)
            nc.vector.tensor_copy(
                pts[pr][hs][:, 0:jn, ts(b, 128)],
                psp[:, ds(0, 128 * jn)].rearrange("p (a b) -> p a b", a=jn))

    def emit_pv(pr, ig):
        """O^T accumulation for row blocks 4*ig..4*ig+3 (both heads)."""
        pos = psA.tile([128, 512], F32, tag="a", name="pos")
        jmax = 4 * ig + 3
        for hs in range(2):
            for j in range(jmax + 1):
                i0 = max(j, 4 * ig)
                col0 = 128 * (i0 - 4 * ig)
                w = 512 - col0
                nc.tensor.matmul(
                    pos[64 * hs:64 * hs + 64, ds(col0, w)],
                    v_sb[:, j, ds(64 * (2 * pr + hs), 64)],
                    pts[pr][hs][:, j, ds(128 * i0, w)],
                    start=(j == 0), stop=(j == jmax),
                    tile_position=(0, 64 * hs),
                    skip_group_check=(j not in (0, jmax)))
        nc.vector.tensor_copy(ot[pr][:, ds(512 * ig, 512)], pos[:, ds(0, 512)])

    ydrain = [0]

    def emit_yproj(nb):
        ysb = work.tile([128, DO], BF16, tag="ysb", name="ysb", bufs=4)
        for oh in range(2):
            psy = psA.tile([128, 512], F32, tag="a", name="psy")
            for pr in range(2):
                nc.tensor.matmul(
                    psy[:, ds(0, 512)], ot[pr][:, ts(nb, 128)],
                    wol[:, pr, ds(512 * oh, 512)],
                    start=(pr == 0), stop=(pr == 1))
            if oh == 0:
                nc.scalar.copy(ysb[:, ts(oh, 512)], psy[:, ds(0, 512)])
            else:
                nc.vector.tensor_copy(ysb[:, ts(oh, 512)], psy[:, ds(0, 512)])
        nc.sync.dma_start(out=ys[ts(nb, 128), :], in_=ysb)

    # ---- phase A: transposes + Q/K h0 projections + e1t ----
    for nb in range(4):
        xpose_block(nb)
    emit_proj(wql, qt, 0, 0, "act")
    emit_proj(wql, qt, 1, 0, "act")

    # E1^T via PE transposes, duplicated to partitions 64..127
    ept = psT.tile([128, 1024], BF16, tag="pt", name="ept")
    for q in range(8):
        nc.tensor.transpose(ept[0:64, ts(q, 128)], e1l[:, q, :], ident)
    nc.vector.tensor_copy(e1t[0:64, :], ept[0:64, :])
    nc.scalar.dma_start(out=e1t[64:128, :], in_=e1t[0:64, :])

    emit_qe(0, 0)
    emit_qe(1, 0)
    for nb in range(4, 8):
        xpose_block(nb)
    emit_qe(0, 1)
    emit_qe(1, 1)
    emit_proj(wkl, kt_sb, 0, 0, "vec")
    emit_proj(wkl, kt_sb, 1, 0, "vec")
    emit_qe(0, 2)
    emit_qe(1, 2)
    emit_qe(0, 3)
    emit_qe(1, 3)

    rels = {}
    # prime the skew-reads for block 0 (their writes are already queued)
    for pr in range(2):
        rel = work.tile([128, 2, N], BF16, tag="rel", name="rel", bufs=4)
        emit_read(pr, 0, rel)
        rels[(pr, 0)] = rel

    # PE filler (projections not yet needed) scheduled into the early,
    # starved steps: (step, pr) -> emit thunks
    filler = {
        (0, 0): [lambda: emit_proj(wql, qt, 0, 1, "act")],
        (0, 1): [lambda: emit_proj(wql, qt, 1, 1, "act"),
                 lambda: emit_v(0)],
        (1, 0): [lambda: emit_proj(wkl, kt_sb, 0, 1, "vec"),
                 lambda: emit_v(1)],
        (1, 1): [lambda: emit_proj(wkl, kt_sb, 1, 1, "vec"),
                 lambda: emit_v(2)],
        (2, 0): [lambda: emit_v(3)],
    }
    # PT(b') emitted at step b: 2-step delay early, 1-step late
    pt_sched = {1: [0], 2: [1], 3: [2], 4: [3], 5: [4], 6: [5, 6]}

    qe_sched = {0: [4, 5], 1: [6, 7]}
    for b in range(NB):
        for pr in range(2):
            flush_deferred()
            emit_s(pr, b, rels.pop((pr, b)))
            for bp in pt_sched.get(b, []):
                emit_pt(pr, bp)
            for f in filler.pop((b, pr), []):
                f()
            for bq in qe_sched.get(b, []):
                emit_qe(pr, bq)
        # issue next block's skew-reads after this block's chain ops
        if b + 1 < NB:
            for pr in range(2):
                rel = work.tile([128, 2, N], BF16, tag="rel", name="rel",
                                bufs=4)
                emit_read(pr, b + 1, rel)
                rels[(pr, b + 1)] = rel
        if b == 5:
            emit_pv(0, 0)
            emit_pv(1, 0)
        if b in (6, 7):
            emit_yproj(2 * (b - 6))
            emit_yproj(2 * (b - 6) + 1)

    flush_deferred()
    emit_pt(0, 7)
    emit_pv(0, 1)
    # pair-0 halves of the tail yprojs overlap pair-1's last PT/PV
    ytail = {}
    for nb in (4, 5):
        for oh in range(2):
            psy = psA.tile([128, 512], F32, tag="a", name="psy")
            nc.tensor.matmul(
                psy[:, ds(0, 512)], ot[0][:, ts(nb, 128)],
                wol[:, 0, ds(512 * oh, 512)], start=True, stop=False)
            ytail[(nb, oh)] = psy
    emit_pt(1, 7)
    emit_pv(1, 1)
    for nb in (4, 5):
        ysb = work.tile([128, DO], BF16, tag="ysb", name="ysb", bufs=4)
        for oh in range(2):
            psy = ytail.pop((nb, oh))
            nc.tensor.matmul(
                psy[:, ds(0, 512)], ot[1][:, ts(nb, 128)],
                wol[:, 1, ds(512 * oh, 512)], start=False, stop=True)
            if oh == 0:
                nc.scalar.copy(ysb[:, ts(oh, 512)], psy[:, ds(0, 512)])
            else:
                nc.vector.tensor_copy(ysb[:, ts(oh, 512)], psy[:, ds(0, 512)])
        nc.sync.dma_start(out=ys[ts(nb, 128), :], in_=ysb)
    for nb in (6, 7):
        emit_yproj(nb)

    ctx.close()


_NC_CACHE = None


def _get_nc():
    global _NC_CACHE
    if _NC_CACHE is None:
        nc = bacc.Bacc(
            "TRN2", target_bir_lowering=False, debug=False, num_devices=8
        )
        with tile.TileContext(nc) as tc:
            _body(tc)
        nc.compile()
        _NC_CACHE = nc
    return _NC_CACHE


def make_in_maps(x, E_rel, Wq, Wk, Wv, Wo):
    import ml_dtypes
    bf = ml_dtypes.bfloat16
    in_maps = []
    for c in range(8):
        b, g = c // 4, c % 4
        cols = slice(COLS * g, COLS * (g + 1))
        in_maps.append({
            "xb": np.ascontiguousarray(x[b]).astype(bf),
            "wq": np.ascontiguousarray(Wq[:, cols]).astype(bf),
            "wk": np.ascontiguousarray(Wk[:, cols]).astype(bf),
            "wv": np.ascontiguousarray(Wv[:, cols]).astype(bf),
            "wo": np.ascontiguousarray(Wo[cols, :]).astype(bf),
            "e1": np.ascontiguousarray(E_rel[:N]).astype(bf),
        })
    return in_maps


def combine(results, bo):
    parts = [np.asarray(results[c]["ys"], dtype=np.float32) for c in range(8)]
    out0 = parts[0] + parts[1] + parts[2] + parts[3] + bo.astype(np.float32)
    out1 = parts[4] + parts[5] + parts[6] + parts[7] + bo.astype(np.float32)
    return np.stack([out0, out1]).astype(np.float32)


def kernel(x, E_rel, mask, Wq, Wk, Wv, Wo, bo, **_):
    nc = _get_nc()
    in_maps = make_in_maps(
        np.asarray(x), np.asarray(E_rel), np.asarray(Wq), np.asarray(Wk),
        np.asarray(Wv), np.asarray(Wo),
    )
    res = run_bass_kernel_spmd(nc, in_maps, list(range(8)))
    return combine(res.results, np.asarray(bo))


# revision 8
# speedup vs baseline: 1.0229x; 1.0229x over previous
"""Multi-head attention with relative position bias (music-transformer skew)
on 8 Trainium2 NeuronCores — v2, instruction/DMA-lean redesign.

Sharding: core c handles batch c//4 and heads 4*(c%4)..+3 (2 pairs of 2
heads). Partial y outputs (bf16) are summed on the host.

Key changes vs v1:
- Padded DRAM scratch per pair [N, 2*1152]: row = [h0 band 1152 | h1 band
  1152], cols 1024..1152 hold 0.0 so the skew-read spill region comes back
  pre-masked (no tri01 multiply, no wrap-chunk writes). One write + one
  skew-read DMA per (pair, block) covering both heads.
- gpsimd cast-DMAs load x/w/e1 as bf16 (half the DMA bytes, no cast ops).
- 1/sqrt(dh) folded into the exp activations via scale=.
- 2-bank [128,1024] PSUM tiles halve drain instruction counts.
- Pairs advance block-by-block in one merged software pipeline so PE never
  drains; p-state stays at 2.4GHz.
- y written once (pairs combined in PSUM) as bf16: 2MB instead of 8MB f32.
"""

import numpy as np

import concourse.bass as bass
import concourse.tile as tile
from concourse import bacc, mybir
from concourse.bass import ds, ts
from concourse.bass_utils import run_bass_kernel_spmd
from concourse.masks import make_identity

B, N, D, H, DH, DO = 2, 1024, 1024, 16, 64, 1024
HPC = 4              # heads per core
COLS = HPC * DH      # 256 projection columns per core
NB = N // 128        # 8 row blocks
KT = D // 128        # 8 contraction tiles
BAND = 1152          # per-head scratch row: 1024 real + 128 pad
RS = 2 * BAND        # scratch row stride (2 heads)
F32 = mybir.dt.float32
BF16 = mybir.dt.bfloat16
SCALE = 1.0 / np.sqrt(DH)
EXP = mybir.ActivationFunctionType.Exp
MULT = mybir.AluOpType.mult


def _body(tc):
    nc = tc.nc
    xb = nc.dram_tensor("xb", [N, D], BF16, kind="ExternalInput")
    wq = nc.dram_tensor("wq", [D, COLS], BF16, kind="ExternalInput")
    wk = nc.dram_tensor("wk", [D, COLS], BF16, kind="ExternalInput")
    wv = nc.dram_tensor("wv", [D, COLS], BF16, kind="ExternalInput")
    wo = nc.dram_tensor("wo", [COLS, DO], BF16, kind="ExternalInput")
    e1 = nc.dram_tensor("e1", [N, DH], BF16, kind="ExternalInput")
    qs = [nc.dram_tensor(f"qs{pr}", [N, RS], BF16) for pr in range(2)]
    ys = nc.dram_tensor("ys", [N, DO], BF16, kind="ExternalOutput")

    from contextlib import ExitStack
    ctx = ExitStack()
    singles = ctx.enter_context(tc.tile_pool(name="singles", bufs=1))
    persist = ctx.enter_context(tc.tile_pool(name="persist", bufs=1))
    work = ctx.enter_context(tc.tile_pool(name="work", bufs=2))
    psA = ctx.enter_context(tc.tile_pool(name="psA", bufs=6, space="PSUM"))
    psT = ctx.enter_context(tc.tile_pool(name="psT", bufs=2, space="PSUM"))

    ident = singles.tile([128, 128], BF16, tag="ident", name="ident")
    make_identity(nc, ident)

    # act-table warmup: load the Exp table while everything else is idle
    warm = singles.tile([128, 1], F32, tag="warm", name="warm")
    nc.scalar.activation(warm, warm, EXP)

    # ---- input loads: gpsimd SWDGE cast-DMAs (f32 DRAM -> bf16 SBUF) ----
    xl = [persist.tile([128, 1, D], BF16, tag=f"xl{q}", name=f"xl{q}")
          for q in range(8)]
    wql = persist.tile([128, KT, COLS], BF16, tag="wql", name="wql")
    e1l = singles.tile([128, 8, DH], BF16, tag="e1l", name="e1l")
    wkl = persist.tile([128, KT, COLS], BF16, tag="wkl", name="wkl")
    wvl = persist.tile([128, KT, COLS], BF16, tag="wvl", name="wvl")
    wol = persist.tile([128, 2, DO], BF16, tag="wol", name="wol")

    for q in range(4):
        nc.sync.dma_start(
            out=xl[q],
            in_=xb.rearrange("(nt p) d -> p nt d", p=128)[:, ds(q, 1), :])
    nc.sync.dma_start(
        out=wql, in_=wq.rearrange("(kt p) c -> p kt c", p=128))
    nc.sync.dma_start(
        out=e1l, in_=e1.rearrange("(nt p) d -> p nt d", p=128))
    for q in range(4, 8):
        nc.sync.dma_start(
            out=xl[q],
            in_=xb.rearrange("(nt p) d -> p nt d", p=128)[:, ds(q, 1), :])
    nc.sync.dma_start(
        out=wkl, in_=wk.rearrange("(kt p) c -> p kt c", p=128))
    nc.sync.dma_start(
        out=wvl, in_=wv.rearrange("(kt p) c -> p kt c", p=128))
    nc.sync.dma_start(
        out=wol, in_=wo.rearrange("(ct p) c -> p ct c", p=128))

    # ---- persistent SBUF ----
    xTt = persist.tile([128, KT, N], BF16, tag="xT", name="xT")
    xT = [xTt[:, k, :] for k in range(KT)]
    qt = [persist.tile([128, N], BF16, tag=f"qt{p}", name=f"qt{p}")
          for p in range(2)]
    kt_sb = [persist.tile([128, N], BF16, tag=f"kt{p}", name=f"kt{p}")
             for p in range(2)]
    v_sb = persist.tile([128, NB, COLS], BF16, tag="vsb", name="vsb")
    e1t = singles.tile([128, N], BF16, tag="e1t", name="e1t")
    ot = [persist.tile([128, N], BF16, tag=f"ot{p}", name=f"ot{p}")
          for p in range(2)]
    pts = [[persist.tile([128, NB, N], BF16, tag=f"pts{p}{h}",
                         name=f"pts{p}{h}") for h in range(2)]
           for p in range(2)]
    # qec staging for the scratch write: manual 3-buffer rotation so the pad
    # columns [1024,1152) can be zeroed exactly once per physical buffer.
    QECB = 4
    qec_l = [persist.tile([128, 2, BAND], BF16, tag=f"qec{b}", name=f"qec{b}")
             for b in range(QECB)]
    for b in range(QECB):
        for hs in range(2):
            nc.vector.memset(qec_l[b][:, hs, ds(N, 128)], -1e9)

    # ---- x transposes (per half: 8 k-chunks packed 2-per-psT-tile) ----
    def xpose_block(nb):
        # 8 transposes (one per k-chunk) for row block nb
        pt_t = psT.tile([128, 1024], BF16, tag="pt", name="pt")
        for k in range(KT):
            nc.tensor.transpose(
                pt_t[:, ts(k, 128)], xl[nb][:, 0, ts(k, 128)], ident)
        nc.vector.tensor_copy(
            xTt[:, :, ts(nb, 128)],
            pt_t.rearrange("p (k c) -> p k c", k=KT))

    # p-state prewarm: dummy transposes so the PE is at full clock when x
    # lands (ramp needs ~3us of continuous busy)
    warmps = psT.tile([128, 1024], BF16, tag="pt", name="warmps")
    for _ in range(12):
        nc.tensor.transpose(warmps[:, ds(0, 128)], ident, ident)

    def emit_proj(w_all, dest, ct, nh, drain):
        ps = psA.tile([128, 512], F32, tag="a", name="psqk")
        for k in range(KT):
            nc.tensor.matmul(
                ps[:, ds(0, 512)], w_all[:, k, ts(ct, 128)],
                xT[k][:, ds(512 * nh, 512)],
                start=(k == 0), stop=(k == KT - 1))
        if drain == "act":
            nc.scalar.copy(dest[ct][:, ds(512 * nh, 512)], ps[:, ds(0, 512)])
        else:
            nc.vector.tensor_copy(
                dest[ct][:, ds(512 * nh, 512)], ps[:, ds(0, 512)])

    def emit_v(mp):
        ps = psA.tile([128, 512], F32, tag="a", name="psv")
        for sub in range(2):
            nb = 2 * mp + sub
            for k in range(KT):
                nc.tensor.matmul(
                    ps[:, ds(256 * sub, 256)],
                    xT[k][:, ts(nb, 128)], wvl[:, k, :],
                    start=(k == 0), stop=(k == KT - 1))
        nc.vector.tensor_copy(
            v_sb[:, ds(2 * mp, 2), :].rearrange("p a b -> p (a b)"),
            ps[:, ds(0, 512)])


    # ---- skew scratch plumbing ----
    def emit_qe(pr, b):
        """expR band for block b, both heads -> qec -> DRAM scratch."""
        lo = 896 - 128 * b
        width = N - lo
        qec = qec_l[qe_ctr[0] % QECB]
        qe_ctr[0] += 1
        nch = (width + 511) // 512
        for hs in range(2):
            base = 64 * hs
            for c in range(nch):
                cw = min(512, width - 512 * c)
                ps = psA.tile([128, 512], F32, tag="a", name="psq")
                nc.tensor.matmul(
                    ps[:, ds(0, cw)],
                    qt[pr][base:base + 64, ts(b, 128)],
                    e1t[base:base + 64, ds(lo + 512 * c, cw)],
                    start=True, stop=True)
                # raw (un-exp'd) drains: hs0 on ACT, hs1 on DVE
                if hs == 1:
                    nc.vector.tensor_copy(
                        qec[:, 1, ds(lo + 512 * c, cw)], ps[:, ds(0, cw)])
                else:
                    nc.scalar.copy(
                        qec[:, 0, ds(lo + 512 * c, cw)], ps[:, ds(0, cw)])
        nc.scalar.dma_start(
            out=bass.AP(
                tensor=qs[pr][:, :].tensor,
                offset=128 * b * RS + lo,
                ap=[[RS, 128], [BAND, 2], [1, BAND - lo]],
            ),
            in_=qec[:, :, ds(lo, BAND - lo)])

    def emit_read(pr, b, rel):
        width = 128 * (b + 1)
        nc.gpsimd.dma_start(
            out=rel[:, :, ds(0, width)],
            in_=bass.AP(
                tensor=qs[pr][:, :].tensor,
                offset=128 * b * (RS - 1) + (N - 1),
                ap=[[RS - 1, 128], [BAND, 2], [1, width]],
            ))

    qe_ctr = [0]
    state = {}
    deferred = []

    def emit_s(pr, b, rel):
        """S matmuls + exp + P=es*rel (+Z) + 1/Z diag for block b."""
        width = 128 * (b + 1)
        nch = (width + 511) // 512
        for hs in range(2):
            base = 64 * hs
            p_sb = work.tile([128, N], BF16, tag=f"p{hs}", name="p", bufs=2)
            zs = []
            for c in range(nch):
                cw = min(512, width - 512 * c)
                ps = psA.tile([128, 512], F32, tag="a", name="pss")
                nc.tensor.matmul(
                    ps[:, ds(0, cw)],
                    qt[pr][base:base + 64, ts(b, 128)],
                    kt_sb[pr][base:base + 64, ds(512 * c, cw)],
                    start=True, stop=False)
                # accumulate the skew-read rel bias into the S PSUM on the
                # PE (identity-matmul add): one fused exp(scale*(S+R)) later
                nc.tensor.matmul(
                    ps[:, ds(0, cw)], ident,
                    rel[:, hs, ds(512 * c, cw)],
                    start=False, stop=True)
                z = work.tile([128, 1], F32, tag=f"z{hs}{c}", name="z",
                              bufs=2)
                nc.scalar.activation(
                    p_sb[:, ds(512 * c, cw)], ps[:, ds(0, cw)],
                    EXP, scale=SCALE, accum_out=z)
                zs.append(z)
            if nch == 2:
                zt = work.tile([128, 1], F32, tag=f"zt{hs}", name="zt",
                               bufs=2)
                nc.vector.tensor_tensor(
                    out=zt, in0=zs[0], in1=zs[1], op=mybir.AluOpType.add)
            else:
                zt = zs[0]
            r = work.tile([128, 1], F32, tag=f"r{hs}", name="r", bufs=2)
            nc.vector.reciprocal(r, zt)
            # normalize rows on DVE (4x perf mode: ~0.26 cyc/elem)
            pn = work.tile([128, N], BF16, tag=f"pn{hs}", name="pn", bufs=3)
            nc.vector.tensor_scalar_mul(
                pn[:, ds(0, width)], p_sb[:, ds(0, width)], r)
            state[(pr, b, hs)] = pn

    def flush_deferred():
        pass

    def emit_pt(pr, b):
        """P^T = P.T @ diag(1/Z) for block b (both heads)."""
        for hs in range(2):
            pn = state.pop((pr, b, hs))
            jn = b + 1
            psp = psT.tile([128, 1024], BF16, tag="pt", name="psp")
            for jj in range(jn):
                nc.tensor.transpose(
                    psp[:, ts(jj, 128)], pn[:, ts(jj, 128)], ident)
            nc.vector.tensor_copy(
                pts[pr][hs][:, 0:jn, ts(b, 128)],
                psp[:, ds(0, 128 * jn)].rearrange("p (a b) -> p a b", a=jn))

    def emit_pv(pr, ig):
        """O^T accumulation for row blocks 4*ig..4*ig+3 (both heads)."""
        pos = psA.tile([128, 512], F32, tag="a", name="pos")
        jmax = 4 * ig + 3
        for hs in range(2):
            for j in range(jmax + 1):
                i0 = max(j, 4 * ig)
                col0 = 128 * (i0 - 4 * ig)
                w = 512 - col0
                nc.tensor.matmul(
                    pos[64 * hs:64 * hs + 64, ds(col0, w)],
                    v_sb[:, j, ds(64 * (2 * pr + hs), 64)],
                    pts[pr][hs][:, j, ds(128 * i0, w)],
                    start=(j == 0), stop=(j == jmax),
                    tile_position=(0, 64 * hs),
                    skip_group_check=(j not in (0, jmax)))
        nc.vector.tensor_copy(ot[pr][:, ds(512 * ig, 512)], pos[:, ds(0, 512)])

    ydrain = [0]

    def emit_yproj(nb):
        ysb = work.tile([128, DO], BF16, tag="ysb", name="ysb", bufs=4)
        for oh in range(2):
            psy = psA.tile([128, 512], F32, tag="a", name="psy")
            for pr in range(2):
                nc.tensor.matmul(
                    psy[:, ds(0, 512)], ot[pr][:, ts(nb, 128)],
                    wol[:, pr, ds(512 * oh, 512)],
                    start=(pr == 0), stop=(pr == 1))
            if oh == 0:
                nc.scalar.copy(ysb[:, ts(oh, 512)], psy[:, ds(0, 512)])
            else:
                nc.vector.tensor_copy(ysb[:, ts(oh, 512)], psy[:, ds(0, 512)])
        nc.sync.dma_start(out=ys[ts(nb, 128), :], in_=ysb)

    # ---- phase A: transposes + Q/K h0 projections + e1t ----
    for nb in range(4):
        xpose_block(nb)
    emit_proj(wql, qt, 0, 0, "act")
    emit_proj(wql, qt, 1, 0, "act")

    # E1^T via PE transposes, duplicated to partitions 64..127
    ept = psT.tile([128, 1024], BF16, tag="pt", name="ept")
    for q in range(8):
        nc.tensor.transpose(ept[0:64, ts(q, 128)], e1l[:, q, :], ident)
    nc.vector.tensor_copy(e1t[0:64, :], ept[0:64, :])
    nc.scalar.dma_start(out=e1t[64:128, :], in_=e1t[0:64, :])

    emit_qe(0, 0)
    emit_qe(1, 0)
    for nb in range(4, 8):
        xpose_block(nb)
    emit_qe(0, 1)
    emit_qe(1, 1)
    emit_proj(wkl, kt_sb, 0, 0, "vec")
    emit_proj(wkl, kt_sb, 1, 0, "vec")
    emit_qe(0, 2)
    emit_qe(1, 2)
    emit_qe(0, 3)
    emit_qe(1, 3)

    rels = {}
    # prime the skew-reads for block 0 (their writes are already queued)
    for pr in range(2):
        rel = work.tile([128, 2, N], BF16, tag="rel", name="rel", bufs=4)
        emit_read(pr, 0, rel)
        rels[(pr, 0)] = rel

    # PE filler (projections not yet needed) scheduled into the early,
    # starved steps: (step, pr) -> emit thunks
    filler = {
        (0, 0): [lambda: emit_proj(wql, qt, 0, 1, "act")],
        (0, 1): [lambda: emit_proj(wql, qt, 1, 1, "act"),
                 lambda: emit_v(0)],
        (1, 0): [lambda: emit_proj(wkl, kt_sb, 0, 1, "vec"),
                 lambda: emit_v(1)],
        (1, 1): [lambda: emit_proj(wkl, kt_sb, 1, 1, "vec"),
                 lambda: emit_v(2)],
        (2, 0): [lambda: emit_v(3)],
    }
    # PT(b') emitted at step b: 2-step delay early, 1-step late
    pt_sched = {1: [0], 2: [1], 3: [2], 4: [3], 5: [4], 6: [5, 6]}

    qe_sched = {0: [4, 5], 1: [6, 7]}
    for b in range(NB):
        for pr in range(2):
            flush_deferred()
            emit_s(pr, b, rels.pop((pr, b)))
            for bp in pt_sched.get(b, []):
                emit_pt(pr, bp)
            for f in filler.pop((b, pr), []):
                f()
            for bq in qe_sched.get(b, []):
                emit_qe(pr, bq)
        # issue next block's skew-reads after this block's chain ops
        if b + 1 < NB:
            for pr in range(2):
                rel = work.tile([128, 2, N], BF16, tag="rel", name="rel",
                                bufs=4)
                emit_read(pr, b + 1, rel)
                rels[(pr, b + 1)] = rel
        if b == 5:
            emit_pv(0, 0)
            emit_pv(1, 0)
        if b in (6, 7):
            emit_yproj(2 * (b - 6))
            emit_yproj(2 * (b - 6) + 1)

    flush_deferred()
    emit_pt(0, 7)
    emit_pv(0, 1)
    # pair-0 halves of the tail yprojs overlap pair-1's last PT/PV
    ytail = {}
    for nb in (4, 5):
        for oh in range(2):
            psy = psA.tile([128, 512], F32, tag="a", name="psy")
            nc.tensor.matmul(
                psy[:, ds(0, 512)], ot[0][:, ts(nb, 128)],
                wol[:, 0, ds(512 * oh, 512)], start=True, stop=False)
            ytail[(nb, oh)] = psy
    emit_pt(1, 7)
    emit_pv(1, 1)
    for nb in (4, 5):
        ysb = work.tile([128, DO], BF16, tag="ysb", name="ysb", bufs=4)
        for oh in range(2):
            psy = ytail.pop((nb, oh))
            nc.tensor.matmul(
                psy[:, ds(0, 512)], ot[1][:, ts(nb, 128)],
                wol[:, 1, ds(512 * oh, 512)], start=False, stop=True)
            if oh == 0:
                nc.scalar.copy(ysb[:, ts(oh, 512)], psy[:, ds(0, 512)])
            else:
                nc.vector.tensor_copy(ysb[:, ts(oh, 512)], psy[:, ds(0, 512)])
        nc.sync.dma_start(out=ys[ts(nb, 128), :], in_=ysb)
    for nb in (6, 7):
        emit_yproj(nb)

    ctx.close()


_NC_CACHE = None


def _get_nc():
    global _NC_CACHE
    if _NC_CACHE is None:
        nc = bacc.Bacc(
            "TRN2", target_bir_lowering=False, debug=False, num_devices=8
        )
        with tile.TileContext(nc) as tc:
            _body(tc)
        nc.compile()
        _NC_CACHE = nc
    return _NC_CACHE


def make_in_maps(x, E_rel, Wq, Wk, Wv, Wo):
    import ml_dtypes
    bf = ml_dtypes.bfloat16
    in_maps = []
    for c in range(8):
        b, g = c // 4, c % 4
        cols = slice(COLS * g, COLS * (g + 1))
        in_maps.append({
            "xb": np.ascontiguousarray(x[b]).astype(bf),
            "wq": np.ascontiguousarray(Wq[:, cols]).astype(bf),
            "wk": np.ascontiguousarray(Wk[:, cols]).astype(bf),
            "wv": np.ascontiguousarray(Wv[:, cols]).astype(bf),
            "wo": np.ascontiguousarray(Wo[cols, :]).astype(bf),
            "e1": np.ascontiguousarray(E_rel[:N]).astype(bf),
        })
    return in_maps


def combine(results, bo):
    parts = [np.asarray(results[c]["ys"], dtype=np.float32) for c in range(8)]
    out0 = parts[0] + parts[1] + parts[2] + parts[3] + bo.astype(np.float32)
    out1 = parts[4] + parts[5] + parts[6] + parts[7] + bo.astype(np.float32)
    return np.stack([out0, out1]).astype(np.float32)


def kernel(x, E_rel, mask, Wq, Wk, Wv, Wo, bo, **_):
    nc = _get_nc()
    in_maps = make_in_maps(
        np.asarray(x), np.asarray(E_rel), np.asarray(Wq), np.asarray(Wk),
        np.asarray(Wv), np.asarray(Wo),
    )
    res = run_bass_kernel_spmd(nc, in_maps, list(range(8)))
    return combine(res.results, np.asarray(bo))
